# revision 44
# baseline (speedup 1.0000x reference)
"""Trainium2 Bass kernel for nn_BlockedMLP (dense_mlp, 8 cores).

Strategy:
  - 8-way data parallel over the batch (B=2048 -> 256 rows/core), weights
    replicated. No collectives.
  - The BSR fc2 (50% block density, 32x32 blocks) is scattered into a dense
    [H, H] matrix on the host: on the PE array a matmul costs N streamed
    columns regardless of contraction K, so 32x32 sparse blocks waste ~4x
    throughput vs dense 128x128 tiles and the block gather costs more than
    the 2x FLOP saving.
  - Feature-major ("transposed") layout throughout: activations live in SBUF
    as [feature_partition, batch_free]; weights are the stationary matmul
    operand, activations stream. Host pre-transposes x and the weights, so
    the device kernel needs no transposes at all.
  - bf16 inputs/weights (host cast) with fp32 PSUM accumulation: 1 cycle/row
    on the PE (fp32 is 4) and half the HBM traffic.
  - Each layer runs as "waves" of 8 output tiles: 8 PSUM banks hold the 8
    accumulators (one accumulation group per bank — a matmul with start=True
    zeroes a whole 2KB zero-region, so groups must not share a bank), the
    k-outer loop streams weight k-tiles [128, 1024] from one packed
    sequential DRAM tensor, and ReLU+bias epilogues run on ScalarE.
"""

import numpy as np
import ml_dtypes

try:
    import concourse.bass as bass  # noqa: F401
except ImportError:
    import sys

    for _p in ("/opt/trn_rl_repo", "/root/.axon_site/_ro/trn_rl_repo"):
        if _p not in sys.path:
            sys.path.insert(0, _p)

import concourse.bacc as bacc
import concourse.bass as bass
import concourse.mybir as mybir
import concourse.tile as tile
from concourse import bass_utils

LIGHT_TAIL = True  # replace Tile's heavy end-of-kernel barrier with a minimal one
FAST_CONST = True  # route Bass-init const-AP memsets to VectorE (GpSimd is ~8x slower)

B, IN, H, OUT, BS = 2048, 1024, 2048, 1024, 32
NCORES = 8
BSH = B // NCORES  # 256 batch rows per core
P = 128
WCOLS = 1024  # streamed weight tile = [P, WCOLS] = 8 output tiles of 128

F32 = mybir.dt.float32
RELU = mybir.ActivationFunctionType.Relu
IDENT = mybir.ActivationFunctionType.Identity

# Wave schedule: (kt, n_out_tiles) per wave; weights packed in this order.
# fc1: 2 waves x 8 k-tiles; fc2: 2 waves x 16; fc3: 1 wave x 16.
NW1, NW2, NW3 = 2, 2, 1
KT1, KT2, KT3 = IN // P, H // P, H // P
WSEQ_TILES = NW1 * KT1 + NW2 * KT2 + NW3 * KT3  # 64

_CACHE = {}


def _emit(tc, DT):
    nc = tc.nc

    xT = nc.dram_tensor("xT", [P, KT1, BSH], DT, kind="ExternalInput").ap()
    wseq = nc.dram_tensor("wseq", [WSEQ_TILES, P, WCOLS], DT, kind="ExternalInput").ap()
    bc = nc.dram_tensor("bc", [P, 2 * H // P + OUT // P], F32, kind="ExternalInput").ap()
    outT = nc.dram_tensor("outT", [OUT // P, P, BSH], F32, kind="ExternalOutput").ap()

    from contextlib import ExitStack

    with ExitStack() as ctx:
        wp = ctx.enter_context(tc.tile_pool(name="wpool", bufs=16))
        act = ctx.enter_context(tc.tile_pool(name="act", bufs=1))
        pp = ctx.enter_context(tc.tile_pool(name="ps", bufs=1, space="PSUM"))
        iop = ctx.enter_context(tc.tile_pool(name="io", bufs=1))

        # x (host-reordered even-k-first) + biases load on the Scalar queue
        # only, so the Sync queue leads with the first weight tile; k=0's
        # x-chunks arrive while w0 streams in parallel.
        xt = iop.tile([P, KT1, BSH], DT, tag="x", name="xt")
        nc.scalar.dma_start(xt[:, 0 : KT1 // 2, :], xT[:, 0 : KT1 // 2, :])
        bs = iop.tile([P, 2 * H // P + OUT // P], F32, tag="bs", name="bs")
        nc.gpsimd.dma_start(bs[:], bc[:])
        nc.gpsimd.dma_start(xt[:, KT1 // 2 : KT1, :], xT[:, KT1 // 2 : KT1, :])
        _xperm = {k: k // 2 if k % 2 == 0 else KT1 // 2 + k // 2 for k in range(KT1)}
        xts = [xt[:, _xperm[k], :] for k in range(KT1)]
        b1s = bs[:, 0 : H // P]
        b2s = bs[:, H // P : 2 * H // P]
        b3s = bs[:, 2 * H // P :]

        # PE warmup: the first real matmul can't start until the first
        # weight tile lands (~12us), guaranteeing a cold (1.2 GHz) PE via
        # the HAM clock gate. Run dummy matmuls on zeroed SBUF during the
        # DMA ramp so the PE is at 2.4 GHz when real work arrives.
        warm_rhs = iop.tile([P, BSH], DT, tag="warm_rhs", name="warm_rhs")
        nc.vector.memset(warm_rhs[:], 0.0)
        warm_ps = pp.tile([P, BSH], F32, tag="ps0", name="warm_ps")
        for i in range(40):
            nc.tensor.matmul(
                warm_ps[:], warm_rhs[:, 0:P], warm_rhs[:], start=True, stop=True
            )

        wslot = [0]  # next tile index in wseq
        # Stripe weight-tile DMAs across independent per-engine HWDGE queues
        # so one queue's slot-semaphore wait doesn't idle all 16 DMA engines.
        dmaq = [nc.sync, nc.scalar]

        wdma = [0]  # weight-DMA instruction counter (for queue striping)

        def wave(kt, rhs_tiles, bias, bias_off, func, out_dt, tag, merge=2):
            """8 out tiles [P, BSH] = func(sum_k w_k.T @ rhs_k + bias).

            merge k-tiles stream per DMA instruction: a dma_start occupies
            the issuing engine ~700ns regardless of size, so fewer+bigger
            wins — except at kernel start, where small first tiles get the
            PE going sooner (merge=1 for the very first wave).
            """
            ps = [
                pp.tile([P, BSH], F32, tag=f"ps{i}", name=f"{tag}ps{i}")
                for i in range(WCOLS // P)
            ]
            for k0 in range(0, kt, merge):
                w = wp.tile([P, merge, WCOLS], DT, tag="w", name=f"{tag}w{k0}")
                src = wseq[wslot[0] : wslot[0] + merge].rearrange("i p c -> p i c")
                dmaq[wdma[0] % 2].dma_start(w[:], src)
                wdma[0] += 1
                wslot[0] += merge
                for kk in range(merge):
                    k = k0 + kk
                    for j in range(WCOLS // P):
                        nc.tensor.matmul(
                            ps[j][:],
                            w[:, kk, j * P : (j + 1) * P],
                            rhs_tiles[k],
                            start=(k == 0),
                            stop=(k == kt - 1),
                        )
            outs = []
            for j in range(WCOLS // P):
                o = act.tile([P, BSH], out_dt, tag=f"{tag}o{j}", name=f"{tag}o{j}")
                bias_ap = bias[:, bias_off + j : bias_off + j + 1]
                # Alternate epilogues between ScalarE and VectorE so the
                # per-wave epilogue chain (which gates PSUM bank reuse by the
                # next wave) halves in length.
                if j % 2 == 0:
                    nc.scalar.activation(o[:], ps[j][:], func, bias=bias_ap)
                elif func is RELU:
                    nc.vector.tensor_scalar(
                        o[:],
                        ps[j][:],
                        bias_ap,
                        0.0,
                        mybir.AluOpType.add,
                        mybir.AluOpType.max,
                    )
                else:
                    nc.vector.tensor_scalar_add(o[:], ps[j][:], bias_ap)
                outs.append(o[:])
            return outs

        hts = []
        for wv in range(NW1):
            hts += wave(KT1, xts, b1s, wv * 8, RELU, DT, f"l1w{wv}", merge=1 if wv == 0 else 2)
        h2s = []
        for wv in range(NW2):
            h2s += wave(KT2, hts, b2s, wv * 8, RELU, DT, f"l2w{wv}")

        # fc3 runs j-outer with all of W3 resident (prefetched while fc2
        # computes): each output tile's epilogue + store overlaps the next
        # tile's matmuls, so only the last tile's epilogue is tail latency.
        w3tiles = []
        for t in range(KT3 // 2):
            w = wp.tile([P, 2, WCOLS], DT, tag="w", name=f"l3w{t}")
            src = wseq[wslot[0] : wslot[0] + 2].rearrange("i p c -> p i c")
            dmaq[wdma[0] % 2].dma_start(w[:], src)
            wdma[0] += 1
            wslot[0] += 2
            w3tiles.append(w)
        for j in range(OUT // P):
            psj = pp.tile([P, BSH], F32, tag=f"ps{j}", name=f"l3ps{j}")
            for k in range(KT3):
                nc.tensor.matmul(
                    psj[:],
                    w3tiles[k // 2][:, k % 2, j * P : (j + 1) * P],
                    h2s[k],
                    start=(k == 0),
                    stop=(k == KT3 - 1),
                )
            o = act.tile([P, BSH], F32, tag=f"l3o{j}", name=f"l3o{j}")
            if j % 2 == 0:
                nc.scalar.activation(o[:], psj[:], IDENT, bias=b3s[:, j : j + 1])
            else:
                nc.vector.tensor_scalar_add(o[:], psj[:], b3s[:, j : j + 1])
            dmaq[j % len(dmaq)].dma_start(outT[j], o[:])


class _LightTailTileContext(tile.TileContext):
    """TileContext with a minimal end-of-kernel sequence.

    Tile's default tail (drain + full all-engine barrier + DMA/semaphore
    reset + second barrier) costs ~8-10us on HW, dominated by NRT's
    expansion of the drain-with-sem-range reset. For a single-TileContext
    kernel the correctness requirement at the end is just: all engines done
    and all output DMAs complete before the NEFF signals completion.
    """

    def _drain_and_barrier(self, tick_clock, wait_clock):
        from concourse.vector_clock import ScopedClock

        drain_inst = self.nc.sync.drain()
        wait_clock.add_sem_waits(
            drain_inst.ins, ScopedClock({None: tick_clock.global_clock})
        )
        self.nc.all_engine_barrier(sem_only=True)
        assert self.sems is not None
        popped = self.nc._tile_sem_poison_stack.pop()
        assert popped is self._sem_poison


def _build(dt_name):
    if dt_name in _CACHE:
        return _CACHE[dt_name]
    DT = {"bf16": mybir.dt.bfloat16, "f32r": mybir.dt.float32r, "f32": F32}[dt_name]

    patches = []
    if FAST_CONST:
        import concourse.bass as cbass

        # During Bass construction only, reroute GpSimd memsets (the
        # framework's const-AP init) to the much faster VectorE: they gate
        # the initial all-engine barrier.
        gps_cls = cbass.BassGpSimd

        def memset_shim(self, ap, constant):
            return self.bass.vector.memset(ap, constant)

        had = "memset" in vars(gps_cls)
        orig = vars(gps_cls).get("memset")
        gps_cls.memset = memset_shim
        patches.append((gps_cls, "memset", had, orig))
        # The barrier after const-AP init protects readers of the const
        # tiles; this kernel never reads them, so skip it.
        bar_orig = cbass.Bass.all_engine_barrier

        def bar_shim(self, *, sem_only=False):
            return None

        cbass.Bass.all_engine_barrier = bar_shim
        patches.append((cbass.Bass, "all_engine_barrier", True, bar_orig))

    try:
        nc = bacc.Bacc(
            "TRN2",
            target_bir_lowering=False,
            debug=False,
            enable_asserts=False,
            num_devices=NCORES,
        )
    finally:
        for klass, attr, had, orig in patches:
            if had:
                setattr(klass, attr, orig)
            else:
                delattr(klass, attr)

    tc_cls = _LightTailTileContext if LIGHT_TAIL else tile.TileContext
    with tc_cls(nc) as tc:
        _emit(tc, DT)
    nc.compile()
    _CACHE[dt_name] = nc
    return nc


def _np_dt(dt_name):
    return mybir.dt.np(
        {"bf16": mybir.dt.bfloat16, "f32r": mybir.dt.float32r, "f32": F32}[dt_name]
    )


def _host_prep(x, W1, b1, crow_indices, col_indices, values, b2, W3, b3, npdt):
    rb = crow_indices.shape[0] - 1
    nnz, bs, _ = values.shape
    cb = H // bs
    # Scatter BSR into dense W2 [H, H].
    blocks = np.zeros((rb, cb, bs, bs), np.float32)
    row_ids = (
        np.searchsorted(crow_indices, np.arange(nnz, dtype=np.int64), side="right") - 1
    )
    blocks[row_ids, col_indices] = values
    W2 = blocks.transpose(0, 2, 1, 3).reshape(H, H)

    # Pack the streamed weight sequence: for each layer, for each wave
    # (column-half), the k-tiles [P, WCOLS] in consumption order.
    def waves(wT, kdim, nw):  # wT [kdim, ndim] -> [nw*kt, P, WCOLS]
        kt = kdim // P
        t = wT.reshape(kt, P, nw, WCOLS).astype(npdt)
        return np.ascontiguousarray(t.transpose(2, 0, 1, 3).reshape(nw * kt, P, WCOLS))

    wseq = np.concatenate(
        [
            waves(np.ascontiguousarray(W1.T), IN, NW1),
            waves(np.ascontiguousarray(W2.T), H, NW2),
            waves(np.ascontiguousarray(W3.T), H, NW3),
        ]
    )
    bc = np.ascontiguousarray(
        np.concatenate(
            [
                b1.reshape(H // P, P).T,
                b2.reshape(H // P, P).T,
                b3.reshape(OUT // P, P).T,
            ],
            axis=1,
        ).astype(np.float32)
    )
    # x -> per-core transposed shards, [P, kt, BSH], k-chunks reordered
    # even-first so each HWDGE queue can load its half in one DMA.
    xT_all = np.ascontiguousarray(x.T.astype(npdt))  # [IN, B]
    korder = [k for k in range(KT1) if k % 2 == 0] + [
        k for k in range(KT1) if k % 2 == 1
    ]
    shards = [
        np.ascontiguousarray(
            xT_all[:, c * BSH : (c + 1) * BSH]
            .reshape(KT1, P, BSH)[korder]
            .transpose(1, 0, 2)
        )
        for c in range(NCORES)
    ]
    shared = dict(wseq=wseq, bc=bc)
    return [dict(shared, xT=shards[c]) for c in range(NCORES)]


def kernel(x, W1, b1, crow_indices, col_indices, values, b2, W3, b3, _dt="bf16"):
    nc = _build(_dt)
    in_maps = _host_prep(
        np.asarray(x, np.float32),
        np.asarray(W1, np.float32),
        np.asarray(b1, np.float32),
        np.asarray(crow_indices),
        np.asarray(col_indices),
        np.asarray(values, np.float32),
        np.asarray(b2, np.float32),
        np.asarray(W3, np.float32),
        np.asarray(b3, np.float32),
        _np_dt(_dt),
    )
    res = bass_utils.run_bass_kernel_spmd(nc, in_maps, core_ids=list(range(NCORES)))
    out = np.concatenate(
        [res.results[c]["outT"].reshape(OUT, BSH).T for c in range(NCORES)], axis=0
    )
    return np.ascontiguousarray(out.astype(np.float32))
